# revision 9
# baseline (speedup 1.0000x reference)
"""Trainium2 Bass kernel: fc1+relu -> LSTM(H=32, T=200) -> fc2 on last hidden.

Data parallel over 8 NeuronCores: batch 4096 -> 512 per core, 4 btiles x 128.

Key structure (B*G layout: batch on partitions for all elementwise work):
  - K-augmentation: per-btile stationary L_k = [h_{t-1}^T (32) | h1aug_t^T (21)]
    (K=53); weights stacked the same way, so ONE matmul per btile computes
    all four gate pre-activations with bias.
  - All gates use tanh (sigmoid(z) = (tanh(z/2)+1)/2; /2 folded into weights
    host-side). Cell kept as C = 2c, hidden as H2 = 2h:
        u = (tf+1)*C      v = (ti+1)*tg     (v on GpSimd, off the DVE)
        C' = 0.5*u + v    tc = tanh(0.5*C')
        H2 = (to+1)*tc
  - Two independent batch streams (btiles {0,1} and {2,3}) interleave their
    serial chains on the engines.
  - fc1 (phase A) computed per 20-step chunk; chunks 0,1 upfront, chunk ci+2
    interleaved one sub-op per timestep into recurrence chunk ci (hides
    phase A and keeps the PE busy).
  - Dummy transposes (const input, spare PSUM bank) are placed before each
    PE burst so the tensor engine p-state stays at mid instead of dropping
    to the 0.65 GHz cold clock between bursts.
"""

import os
import sys
import numpy as np
from contextlib import ExitStack

sys.path.insert(0, "/opt/trn_rl_repo")
sys.path.insert(0, "/opt/pypackages")

import concourse.bass as bass
import concourse.bacc as bacc
import concourse.tile as tile
import concourse.mybir as mybir
from concourse import bass_utils
from concourse.masks import make_identity

F32 = mybir.dt.float32
BF16 = mybir.dt.bfloat16
AF = mybir.ActivationFunctionType
ALU = mybir.AluOpType

H = 32
B = 4096
T = 200
CIN = 5
C6 = 6
NCORES = 8
BL = B // NCORES  # 512
NBT = BL // 128  # 4
TCH = 20  # timesteps per chunk
NCH = T // TCH  # 10
QW = 64  # per-(t,btile) block width in Q: [H2(32) | h1aug(21) | pad(11)]
QROW = NBT * QW  # 256 per timestep

# gate blocks: 0=f, 1=i, 2=g, 3=o ; torch rows i,f,g,o
_TORCH_BASE = {0: 32, 1: 0, 2: 64, 3: 96}


def _perm_scale():
    perm = np.zeros(4 * H, dtype=np.int64)
    srow = np.zeros(4 * H, dtype=np.float32)
    for j in range(4 * H):
        blk, idx = j // H, j % H
        perm[j] = _TORCH_BASE[blk] + idx
        srow[j] = 1.0 if blk == 2 else 0.5
    return perm, srow


def prep_consts(fc1_w, fc1_b, w_ih, w_hh, b_ih, b_hh, fc2_w, fc2_b):
    perm, srow = _perm_scale()
    wcomb = np.zeros((53, 128), np.float32)
    wcomb[0:32] = 0.5 * (srow[:, None] * w_hh[perm]).T
    wcomb[32:52] = (srow[:, None] * w_ih[perm]).T
    wcomb[52] = srow * (b_ih + b_hh)[perm]
    # w1bd [120, 21*TCH]: block-diagonal fc1 (+bias via c=5 row, ones col 20)
    w1bd = np.zeros((C6 * TCH, 21 * TCH), np.float32)
    for w in range(TCH):
        for c in range(CIN):
            w1bd[C6 * w + c, 21 * w : 21 * w + 20] = fc1_w[:, c]
        w1bd[C6 * w + CIN, 21 * w : 21 * w + 20] = fc1_b
        w1bd[C6 * w + CIN, 21 * w + 20] = 1.0
    fc2w_rep = np.ascontiguousarray(0.5 * fc2_w.T)  # [32,2]
    import ml_dtypes

    bf = ml_dtypes.bfloat16
    return dict(
        wcomb=wcomb.astype(bf), w1bd=w1bd.astype(bf), fc2w_rep=fc2w_rep.astype(bf)
    )


def emit(tc, outs, ins):
    nc = tc.nc
    ctx = ExitStack()
    xd = ins["x"]  # [512, 1000]
    out_d = outs["out"]  # [512, 2]

    _interleave = os.environ.get("K_INTERLEAVE", "1") == "1"
    _dummies = os.environ.get("K_DUMMIES", "1") == "1"
    _vgp = os.environ.get("K_VGP", "1") == "1"

    consts = ctx.enter_context(tc.tile_pool(name="consts", bufs=1))
    ident = consts.tile([128, 128], BF16, tag="ident")
    make_identity(nc, ident[:])
    wcomb = consts.tile([53, 128], BF16, tag="wcomb")
    nc.sync.dma_start(wcomb[:], ins["wcomb"][:, :])
    w1bd = consts.tile([C6 * TCH, 21 * TCH], BF16, tag="w1bd")
    nc.sync.dma_start(w1bd[:], ins["w1bd"][:, :])
    fc2w = consts.tile([32, 2], BF16, tag="fc2w")
    nc.sync.dma_start(fc2w[:], ins["fc2w_rep"][:, :])

    # ---------------- pools ----------------
    xpool = ctx.enter_context(tc.tile_pool(name="x6", bufs=1))
    xsb_pool = ctx.enter_context(tc.tile_pool(name="xsb", bufs=2))
    psum = ctx.enter_context(tc.tile_pool(name="ps", bufs=1, space="PSUM"))
    xt_pool = ctx.enter_context(tc.tile_pool(name="xt", bufs=2))
    q_pool = ctx.enter_context(tc.tile_pool(name="q", bufs=4))
    st_pool = ctx.enter_context(tc.tile_pool(name="st", bufs=1))
    work = ctx.enter_context(tc.tile_pool(name="wk", bufs=2))

    # ---------------- x load, pad ----------------
    x6 = [
        xpool.tile([128, C6 * T], BF16, tag=f"x6_{k}", name=f"x6_{k}")
        for k in range(NBT)
    ]
    for k in range(NBT):
        xs = xsb_pool.tile([128, CIN * T], F32, tag="xsb", name=f"xs_{k}")
        nc.sync.dma_start(xs[:], xd[128 * k : 128 * (k + 1), :])
        nc.gpsimd.memset(x6[k][:], 1.0)
        nc.vector.tensor_copy(
            x6[k][:].rearrange("p (t c) -> p t c", c=C6)[:, :, 0:CIN],
            xs[:].rearrange("p (t c) -> p t c", c=CIN),
        )

    qf = q_pool.tile([128, QROW], BF16, tag="qf", bufs=1)
    nc.vector.memset(qf[:], 0.0)

    qcs = {}

    # PE-warming dummy: a tiny transpose whose input DEPENDS on late-chain
    # data (tct), so it executes right before the next step's transpose
    # burst and keeps the tensor-engine p-state out of the cold clock.
    dum_ps = psum.tile([128, 256], BF16, tag="dum", bufs=1, name="dum_ps") if _dummies else None

    def dummy(src_ap, s):
        if not _dummies:
            return
        nc.tensor.transpose(dum_ps[0:8, 128 * s : 128 * s + 128], src_ap, ident[:])

    # ---------------- phase A (per chunk) as thunk list ----------------
    def phase_a_thunks(ci):
        th = []
        box = {}

        def alloc():
            qt = q_pool.tile([128, TCH * QROW], BF16, tag="qc", name=f"qc_{ci}")
            qcs[ci] = qt
            if ci == 0:
                nc.vector.memset(qt[:, 0:QROW], 0.0)
            xtp = psum.tile([C6 * TCH, 512], BF16, tag="xtp", bufs=1, name=f"xtp_{ci}")
            box["xtp"] = xtp
            nc.tensor.transpose(
                xtp[:, 0:128], x6[0][:, C6 * TCH * ci : C6 * TCH * (ci + 1)], ident[:]
            )

        th.append(alloc)

        def tr(k):
            def f():
                nc.tensor.transpose(
                    box["xtp"][:, 128 * k : 128 * (k + 1)],
                    x6[k][:, C6 * TCH * ci : C6 * TCH * (ci + 1)],
                    ident[:],
                )

            return f

        for k in range(1, NBT):
            th.append(tr(k))

        def cp():
            xt = xt_pool.tile([C6 * TCH, 512], BF16, tag="xt", name=f"xt_{ci}")
            box["xt"] = xt
            nc.vector.tensor_copy(xt[:], box["xtp"][:])

        th.append(cp)

        def mk_mm(k):
            def f():
                fps = psum.tile(
                    [128, 21 * TCH], F32, tag="fc1", bufs=1, name=f"fps_{ci}_{k}"
                )
                box[f"fps{k}"] = fps
                nc.tensor.matmul(
                    fps[:],
                    box["xt"][:, 128 * k : 128 * (k + 1)],
                    w1bd[:],
                    start=True,
                    stop=True,
                    tile_position=(0, 0),
                )

            return f

        def mk_relu(k):
            def f():
                qv = qcs[ci][:].rearrange("p (w b) -> p w b", b=QROW)
                nc.scalar.activation(
                    qv[:, :, QW * k + 32 : QW * k + 53],
                    box[f"fps{k}"][:].rearrange("p (w m) -> p w m", m=21),
                    AF.Relu,
                )

            return f

        for k in range(NBT):
            th.append(mk_mm(k))
            th.append(mk_relu(k))
        return th

    n_upfront = 2 if _interleave else NCH
    for ci in range(n_upfront):
        for f in phase_a_thunks(ci):
            f()

    # ---------------- Phase B: recurrence ----------------
    _stage = int(os.environ.get("K_STAGE", "9"))
    Cst = psum.tile([128, 128], F32, tag="C", bufs=1, name="Cst")
    nc.vector.memset(Cst[:], 0.0)

    SB = ("a", "b")
    for ci in range(NCH):
        th = (
            phase_a_thunks(ci + 2)
            if (_interleave and ci + 2 < NCH and _stage >= 2)
            else []
        )
        for w in range(TCH):
            t = ci * TCH + w
            if _stage >= 2:
                if t + 1 < T:
                    cin, wn = (t + 1) // TCH, (t + 1) % TCH
                    qdst = qcs[cin][:, QROW * wn : QROW * (wn + 1)]
                else:
                    qdst = qf[:]
                qv4 = qdst.rearrange("p (k s) -> p k s", s=QW)
                for s in range(2):
                    sb = SB[s]
                    tp = psum.tile(
                        [53, 256], BF16, tag=f"tp{sb}", bufs=1, name=f"tp{sb}_{t}"
                    )
                    for j in range(2):
                        k = 2 * s + j
                        nc.tensor.transpose(
                            tp[:, 128 * j : 128 * (j + 1)],
                            qcs[ci][:, QROW * w + QW * k : QROW * w + QW * k + 53],
                            ident[:],
                        )
                    L = work.tile([53, 256], BF16, tag=f"L{sb}", name=f"L{sb}_{t}")
                    nc.vector.tensor_copy(L[:, 0:128], tp[:, 0:128])
                    nc.vector.tensor_copy(L[:, 128:256], tp[:, 128:256])
                    if _stage < 3:
                        continue
                    gt = psum.tile(
                        [128, 256], F32, tag=f"g{sb}", bufs=1, name=f"g{sb}_{t}"
                    )
                    for j in range(2):
                        nc.tensor.matmul(
                            gt[:, 128 * j : 128 * (j + 1)],
                            L[:, 128 * j : 128 * (j + 1)],
                            wcomb[:],
                            start=True,
                            stop=True,
                            tile_position=(0, 0),
                        )
                    if _stage < 4:
                        continue
                    t4 = work.tile([128, 256], BF16, tag=f"t4{sb}", name=f"t4{sb}_{t}")
                    g4 = t4[:].rearrange("p (k g) -> p k g", k=2)
                    gv = gt[:].rearrange("p (k g) -> p k g", k=2)
                    nc.scalar.activation(g4[:, :, 0:96], gv[:, :, 0:96], AF.Tanh)
                    nc.scalar.activation(g4[:, :, 96:128], gv[:, :, 96:128], AF.Tanh)
                    tf, ti = g4[:, :, 0:32], g4[:, :, 32:64]
                    tg, to = g4[:, :, 64:96], g4[:, :, 96:128]
                    Cs = Cst[:, 64 * s : 64 * (s + 1)]
                    if _stage < 5:
                        continue
                    ut = work.tile([128, 64], F32, tag=f"u{sb}", name=f"u{sb}_{t}")
                    nc.vector.scalar_tensor_tensor(ut[:], tf, 1.0, Cs, ALU.add, ALU.mult)
                    vt = work.tile([128, 64], F32, tag=f"v{sb}", name=f"v{sb}_{t}")
                    nc.vector.scalar_tensor_tensor(vt[:], ti, 1.0, tg, ALU.add, ALU.mult)
                    nc.vector.scalar_tensor_tensor(Cs, ut[:], 0.5, vt[:], ALU.mult, ALU.add)
                    if _stage < 6:
                        continue
                    tct = work.tile([128, 64], BF16, tag=f"tc{sb}", name=f"tc{sb}_{t}")
                    nc.scalar.activation(tct[:], Cs, AF.Tanh, scale=0.5)
                    dummy(tct[:, 0:8], s)
                    tcv = tct[:].rearrange("p (k c) -> p k c", c=32)
                    for j in range(2):
                        nc.vector.scalar_tensor_tensor(
                            qv4[:, 2 * s + j : 2 * s + j + 1, 0:32],
                            g4[:, j : j + 1, 96:128],
                            1.0,
                            tcv[:, j : j + 1, :],
                            ALU.add,
                            ALU.mult,
                        )
            # interleaved phase A sub-op for chunk ci+2
            if w < len(th):
                th[w]()

    # ---------------- fc2 ----------------
    f2p = psum.tile([128, 8], F32, tag="xtp", bufs=1, name="f2p")
    for s in range(2):
        sb = SB[s]
        tpf = psum.tile([53, 256], BF16, tag=f"tp{sb}", bufs=1, name=f"tpf{sb}")
        for j in range(2):
            k = 2 * s + j
            nc.tensor.transpose(
                tpf[0:32, 128 * j : 128 * (j + 1)],
                qf[:, QW * k : QW * k + 32],
                ident[:],
            )
        Lf = work.tile([53, 256], BF16, tag=f"L{sb}", name=f"Lf{sb}")
        nc.vector.tensor_copy(Lf[0:32, :], tpf[0:32, :])
        for j in range(2):
            k = 2 * s + j
            nc.tensor.matmul(
                f2p[:, 2 * k : 2 * k + 2],
                Lf[0:32, 128 * j : 128 * (j + 1)],
                fc2w[:],
                start=True,
                stop=True,
                tile_position=(0, 0),
            )
    f2s = work.tile([128, 8], F32, tag="f2s", name="f2s")
    nc.vector.tensor_copy(f2s[:], f2p[:])
    for k in range(NBT):
        nc.sync.dma_start(
            out_d[128 * k : 128 * (k + 1), :], f2s[:, 2 * k : 2 * k + 2]
        )
    ctx.close()


_CACHE = {}


def _build():
    if "nc" in _CACHE:
        return _CACHE["nc"]
    nc = bacc.Bacc(
        "TRN2",
        target_bir_lowering=False,
        debug=False,
        enable_asserts=False,
        num_devices=NCORES,
    )
    ins = {
        "x": nc.dram_tensor("x", [BL, CIN * T], F32, kind="ExternalInput").ap(),
        "wcomb": nc.dram_tensor("wcomb", [53, 128], BF16, kind="ExternalInput").ap(),
        "w1bd": nc.dram_tensor(
            "w1bd", [C6 * TCH, 21 * TCH], BF16, kind="ExternalInput"
        ).ap(),
        "fc2w_rep": nc.dram_tensor(
            "fc2w_rep", [32, 2], BF16, kind="ExternalInput"
        ).ap(),
    }
    outs = {"out": nc.dram_tensor("out", [BL, 2], F32, kind="ExternalOutput").ap()}
    with tile.TileContext(nc) as tc:
        emit(tc, outs, ins)
    nc.compile()
    _CACHE["nc"] = nc
    return nc


def make_in_maps(x, fc1_w, fc1_b, w_ih, w_hh, b_ih, b_hh, fc2_w, fc2_b):
    consts = prep_consts(fc1_w, fc1_b, w_ih, w_hh, b_ih, b_hh, fc2_w, fc2_b)
    in_maps = []
    for c in range(NCORES):
        xs = np.ascontiguousarray(
            x[c * BL : (c + 1) * BL].reshape(BL, CIN * T)
        ).astype(np.float32)
        in_maps.append({"x": xs, **consts})
    return in_maps


def kernel(x, fc1_w, fc1_b, w_ih, w_hh, b_ih, b_hh, fc2_w, fc2_b, trace=False):
    x = np.asarray(x, np.float32)
    args = [
        np.asarray(a, np.float32)
        for a in (fc1_w, fc1_b, w_ih, w_hh, b_ih, b_hh, fc2_w, fc2_b)
    ]
    nc = _build()
    in_maps = make_in_maps(x, *args)
    res = bass_utils.run_bass_kernel_spmd(
        nc, in_maps, core_ids=list(range(NCORES)), trace=trace
    )
    out = np.concatenate([r["out"] for r in res.results], axis=0)
    out = out + args[7][None, :]
    if trace:
        kernel.last_results = res
    return out.astype(np.float32)


# revision 13
# speedup vs baseline: 1.4974x; 1.4974x over previous
"""Trainium2 Bass kernel: fc1+relu -> LSTM(H=32, T=200) -> fc2 on last hidden.

Data parallel over 8 NeuronCores: batch 4096 -> 512 per core, 4 btiles x 128.

Key structure (B*G layout: batch on partitions for all elementwise work):
  - K-augmentation: per-btile stationary L_k = [h_{t-1}^T (32) | h1aug_t^T (21)]
    (K=53); weights stacked the same way, so ONE matmul per btile computes
    all four gate pre-activations with bias.
  - All gates use tanh (sigmoid(z) = (tanh(z/2)+1)/2; /2 folded into weights
    host-side). Cell kept as C = 2c, hidden as H2 = 2h:
        u = (tf+1)*C      v = (ti+1)*tg     (v on GpSimd, off the DVE)
        C' = 0.5*u + v    tc = tanh(0.5*C')
        H2 = (to+1)*tc
  - Two independent batch streams (btiles {0,1} and {2,3}) interleave their
    serial chains on the engines.
  - fc1 (phase A) computed per 20-step chunk; chunks 0,1 upfront, chunk ci+2
    interleaved one sub-op per timestep into recurrence chunk ci (hides
    phase A and keeps the PE busy).
  - Dummy transposes (const input, spare PSUM bank) are placed before each
    PE burst so the tensor engine p-state stays at mid instead of dropping
    to the 0.65 GHz cold clock between bursts.
"""

import os
import sys
import numpy as np
from contextlib import ExitStack

sys.path.insert(0, "/opt/trn_rl_repo")
sys.path.insert(0, "/opt/pypackages")

import concourse.bass as bass
import concourse.bacc as bacc
import concourse.tile as tile
import concourse.mybir as mybir
from concourse import bass_utils
from concourse.masks import make_identity

F32 = mybir.dt.float32
F16 = mybir.dt.float16
BF16 = mybir.dt.bfloat16
AF = mybir.ActivationFunctionType
ALU = mybir.AluOpType

H = 32
B = 4096
T = 200
CIN = 5
C6 = 6
NCORES = 8
BL = B // NCORES  # 512
NBT = BL // 128  # 4
TCH = 20  # timesteps per chunk
NCH = T // TCH  # 10
QW = 64  # per-(t,btile) block width in Q: [H2(32) | h1aug(21) | pad(11)]
QROW = NBT * QW  # 256 per timestep

# gate blocks: 0=o, 1=i, 2=f, 3=g ; torch rows i,f,g,o
_TORCH_BASE = {0: 96, 1: 0, 2: 32, 3: 64}


def _perm_scale():
    perm = np.zeros(4 * H, dtype=np.int64)
    srow = np.zeros(4 * H, dtype=np.float32)
    for j in range(4 * H):
        blk, idx = j // H, j % H
        perm[j] = _TORCH_BASE[blk] + idx
        srow[j] = 1.0 if blk == 3 else 0.5
    return perm, srow


def prep_consts(fc1_w, fc1_b, w_ih, w_hh, b_ih, b_hh, fc2_w, fc2_b):
    perm, srow = _perm_scale()
    wcomb = np.zeros((53, 128), np.float32)
    wcomb[0:32] = 0.5 * (srow[:, None] * w_hh[perm]).T
    wcomb[32:52] = (srow[:, None] * w_ih[perm]).T
    wcomb[52] = srow * (b_ih + b_hh)[perm]
    # w1bd [120, 21*TCH]: block-diagonal fc1 (+bias via c=5 row, ones col 20)
    w1bd = np.zeros((C6 * TCH, 21 * TCH), np.float32)
    for w in range(TCH):
        for c in range(CIN):
            w1bd[C6 * w + c, 21 * w : 21 * w + 20] = fc1_w[:, c]
        w1bd[C6 * w + CIN, 21 * w : 21 * w + 20] = fc1_b
        w1bd[C6 * w + CIN, 21 * w + 20] = 1.0
    fc2w_rep = np.ascontiguousarray(0.5 * fc2_w.T)  # [32,2]
    import ml_dtypes

    bf = ml_dtypes.bfloat16
    return dict(
        wcomb=wcomb.astype(bf), w1bd=w1bd.astype(bf), fc2w_rep=fc2w_rep.astype(bf)
    )


def emit(tc, outs, ins):
    nc = tc.nc
    ctx = ExitStack()
    xd = ins["x"]  # [512, 1000]
    out_d = outs["out"]  # [512, 2]

    _interleave = os.environ.get("K_INTERLEAVE", "1") == "1"
    _dummies = os.environ.get("K_DUMMIES", "0") == "1"
    _vgp = os.environ.get("K_VGP", "1") == "1"

    consts = ctx.enter_context(tc.tile_pool(name="consts", bufs=1))
    ident = consts.tile([128, 128], BF16, tag="ident")
    make_identity(nc, ident[:])
    wcomb = consts.tile([53, 128], BF16, tag="wcomb")
    nc.sync.dma_start(wcomb[:], ins["wcomb"][:, :])
    w1bd = consts.tile([C6 * TCH, 21 * TCH], BF16, tag="w1bd")
    nc.sync.dma_start(w1bd[:], ins["w1bd"][:, :])
    fc2w = consts.tile([32, 2], BF16, tag="fc2w")
    nc.sync.dma_start(fc2w[:], ins["fc2w_rep"][:, :])

    # ---------------- pools ----------------
    xpool = ctx.enter_context(tc.tile_pool(name="x6", bufs=1))
    xsb_pool = ctx.enter_context(tc.tile_pool(name="xsb", bufs=2))
    psum = ctx.enter_context(tc.tile_pool(name="ps", bufs=1, space="PSUM"))
    xt_pool = ctx.enter_context(tc.tile_pool(name="xt", bufs=2))
    q_pool = ctx.enter_context(tc.tile_pool(name="q", bufs=4))
    st_pool = ctx.enter_context(tc.tile_pool(name="st", bufs=1))
    work = ctx.enter_context(tc.tile_pool(name="wk", bufs=2))

    # ---------------- x load, pad ----------------
    x6 = [
        xpool.tile([128, C6 * T], BF16, tag=f"x6_{k}", name=f"x6_{k}")
        for k in range(NBT)
    ]
    for k in range(NBT):
        xs = xsb_pool.tile([128, CIN * T], F32, tag="xsb", name=f"xs_{k}")
        nc.sync.dma_start(xs[:], xd[128 * k : 128 * (k + 1), :])
        nc.gpsimd.memset(x6[k][:], 1.0)
        nc.vector.tensor_copy(
            x6[k][:].rearrange("p (t c) -> p t c", c=C6)[:, :, 0:CIN],
            xs[:].rearrange("p (t c) -> p t c", c=CIN),
        )

    qf = q_pool.tile([128, QROW], BF16, tag="qf", bufs=1)
    nc.vector.memset(qf[:], 0.0)

    qcs = {}

    # PE-warming dummy: a tiny transpose whose input DEPENDS on late-chain
    # data (tct), so it executes right before the next step's transpose
    # burst and keeps the tensor-engine p-state out of the cold clock.
    dum_ps = psum.tile([128, 256], BF16, tag="dum", bufs=1, name="dum_ps") if _dummies else None

    def dummy(src_ap, s):
        if not _dummies:
            return
        nc.tensor.transpose(dum_ps[0:8, 128 * s : 128 * s + 128], src_ap, ident[:])

    # ---------------- phase A (per chunk) as thunk list ----------------
    def phase_a_thunks(ci):
        th = []
        box = {}

        def alloc():
            qt = q_pool.tile([128, TCH * QROW], BF16, tag="qc", name=f"qc_{ci}")
            qcs[ci] = qt
            if ci == 0:
                nc.vector.memset(qt[:, 0:QROW], 0.0)
            xtp = psum.tile([C6 * TCH, 512], BF16, tag="xtp", bufs=1, name=f"xtp_{ci}")
            box["xtp"] = xtp
            nc.tensor.transpose(
                xtp[:, 0:128], x6[0][:, C6 * TCH * ci : C6 * TCH * (ci + 1)], ident[:]
            )

        th.append(alloc)

        def tr(k):
            def f():
                nc.tensor.transpose(
                    box["xtp"][:, 128 * k : 128 * (k + 1)],
                    x6[k][:, C6 * TCH * ci : C6 * TCH * (ci + 1)],
                    ident[:],
                )

            return f

        for k in range(1, NBT):
            th.append(tr(k))

        def cp():
            xt = xt_pool.tile([C6 * TCH, 512], BF16, tag="xt", name=f"xt_{ci}")
            box["xt"] = xt
            nc.vector.tensor_copy(xt[:], box["xtp"][:])

        th.append(cp)

        def mk_mm(k):
            def f():
                fps = psum.tile(
                    [128, 21 * TCH], F32, tag="fc1", bufs=1, name=f"fps_{ci}_{k}"
                )
                box[f"fps{k}"] = fps
                nc.tensor.matmul(
                    fps[:],
                    box["xt"][:, 128 * k : 128 * (k + 1)],
                    w1bd[:],
                    start=True,
                    stop=True,
                    tile_position=(0, 0),
                )

            return f

        def mk_relu(k):
            def f():
                qv = qcs[ci][:].rearrange("p (w b) -> p w b", b=QROW)
                nc.scalar.activation(
                    qv[:, :, QW * k + 32 : QW * k + 53],
                    box[f"fps{k}"][:].rearrange("p (w m) -> p w m", m=21),
                    AF.Relu,
                )

            return f

        for k in range(NBT):
            th.append(mk_mm(k))
            th.append(mk_relu(k))
        return th

    n_upfront = 2 if _interleave else NCH
    for ci in range(n_upfront):
        for f in phase_a_thunks(ci):
            f()

    # ---------------- Phase B: recurrence ----------------
    _stage = int(os.environ.get("K_STAGE", "9"))
    # per-stream state/gate tile: two 256-col blocks (one per btile),
    # block layout [o(32) i(32) f(32) g(32) C(32) pad(96)] -- power-of-2
    # stride keeps strided APs on the fast path; C = 2*cell persists.
    X4 = [
        st_pool.tile([128, 512], F16, tag=f"X4{u}", name=f"X4{u}") for u in ("a", "b")
    ]
    for xt_ in X4:
        nc.vector.memset(xt_[:], 0.0)

    SB = ("a", "b")
    for ci in range(NCH):
        th = (
            phase_a_thunks(ci + 2)
            if (_interleave and ci + 2 < NCH and _stage >= 2)
            else []
        )
        for w in range(TCH):
            t = ci * TCH + w
            if _stage >= 2:
                if t + 1 < T:
                    cin, wn = (t + 1) // TCH, (t + 1) % TCH
                    qdst = qcs[cin][:, QROW * wn : QROW * (wn + 1)]
                else:
                    qdst = qf[:]
                qv4 = qdst.rearrange("p (k s) -> p k s", s=QW)
                for s in range(2):
                    sb = SB[s]
                    tp = psum.tile(
                        [53, 256], BF16, tag=f"tp{sb}", bufs=1, name=f"tp{sb}_{t}"
                    )
                    for j in range(2):
                        k = 2 * s + j
                        nc.tensor.transpose(
                            tp[:, 128 * j : 128 * (j + 1)],
                            qcs[ci][:, QROW * w + QW * k : QROW * w + QW * k + 53],
                            ident[:],
                        )
                    L = work.tile([53, 256], BF16, tag=f"L{sb}", name=f"L{sb}_{t}")
                    nc.vector.tensor_copy(L[:], tp[:])
                    if _stage < 3:
                        continue
                    gt = psum.tile(
                        [128, 256], F32, tag=f"g{sb}", bufs=1, name=f"g{sb}_{t}"
                    )
                    for j in range(2):
                        nc.tensor.matmul(
                            gt[:, 128 * j : 128 * (j + 1)],
                            L[:, 128 * j : 128 * (j + 1)],
                            wcomb[:],
                            start=True,
                            stop=True,
                            tile_position=(0, 0),
                        )
                    if _stage < 4:
                        continue
                    Xs = X4[s][:].rearrange("p (k c) -> p k c", c=256)
                    nc.scalar.activation(
                        Xs[:, :, 0:128],
                        gt[:].rearrange("p (k g) -> p k g", k=2),
                        AF.Tanh,
                    )
                    if _stage < 5:
                        continue
                    UV = work.tile([128, 128], F16, tag=f"uv{sb}", name=f"uv{sb}_{t}")
                    UVv = UV[:].rearrange("p (k c) -> p k c", c=64)
                    # [v|u] = ([i|f] + 1) * [g|C]
                    nc.vector.scalar_tensor_tensor(
                        UVv[:, :, :],
                        Xs[:, :, 32:96],
                        1.0,
                        Xs[:, :, 96:160],
                        ALU.add,
                        ALU.mult,
                    )
                    # C' = 0.5*u + v (into the C slot)
                    nc.vector.scalar_tensor_tensor(
                        Xs[:, :, 128:160],
                        UVv[:, :, 32:64],
                        0.5,
                        UVv[:, :, 0:32],
                        ALU.mult,
                        ALU.add,
                    )
                    if _stage < 6:
                        continue
                    tct = work.tile([128, 64], F16, tag=f"tc{sb}", name=f"tc{sb}_{t}")
                    tcv = tct[:].rearrange("p (k c) -> p k c", c=32)
                    nc.scalar.activation(tcv[:, :, :], Xs[:, :, 128:160], AF.Tanh, scale=0.5)
                    nc.vector.scalar_tensor_tensor(
                        qv4[:, 2 * s : 2 * s + 2, 0:32],
                        Xs[:, :, 0:32],
                        1.0,
                        tcv[:, :, :],
                        ALU.add,
                        ALU.mult,
                    )
            # interleaved phase A sub-op for chunk ci+2
            if w < len(th):
                th[w]()

    # ---------------- fc2 ----------------
    f2p = psum.tile([128, 8], F32, tag="xtp", bufs=1, name="f2p")
    for s in range(2):
        sb = SB[s]
        tpf = psum.tile([53, 256], BF16, tag=f"tp{sb}", bufs=1, name=f"tpf{sb}")
        for j in range(2):
            k = 2 * s + j
            nc.tensor.transpose(
                tpf[0:32, 128 * j : 128 * (j + 1)],
                qf[:, QW * k : QW * k + 32],
                ident[:],
            )
        Lf = work.tile([53, 256], BF16, tag=f"L{sb}", name=f"Lf{sb}")
        nc.vector.tensor_copy(Lf[0:32, :], tpf[0:32, :])
        for j in range(2):
            k = 2 * s + j
            nc.tensor.matmul(
                f2p[:, 2 * k : 2 * k + 2],
                Lf[0:32, 128 * j : 128 * (j + 1)],
                fc2w[:],
                start=True,
                stop=True,
                tile_position=(0, 0),
            )
    f2s = work.tile([128, 8], F32, tag="f2s", name="f2s")
    nc.vector.tensor_copy(f2s[:], f2p[:])
    for k in range(NBT):
        nc.sync.dma_start(
            out_d[128 * k : 128 * (k + 1), :], f2s[:, 2 * k : 2 * k + 2]
        )
    ctx.close()


_CACHE = {}


def _build():
    if "nc" in _CACHE:
        return _CACHE["nc"]
    nc = bacc.Bacc(
        "TRN2",
        target_bir_lowering=False,
        debug=False,
        enable_asserts=False,
        num_devices=NCORES,
    )
    ins = {
        "x": nc.dram_tensor("x", [BL, CIN * T], F32, kind="ExternalInput").ap(),
        "wcomb": nc.dram_tensor("wcomb", [53, 128], BF16, kind="ExternalInput").ap(),
        "w1bd": nc.dram_tensor(
            "w1bd", [C6 * TCH, 21 * TCH], BF16, kind="ExternalInput"
        ).ap(),
        "fc2w_rep": nc.dram_tensor(
            "fc2w_rep", [32, 2], BF16, kind="ExternalInput"
        ).ap(),
    }
    outs = {"out": nc.dram_tensor("out", [BL, 2], F32, kind="ExternalOutput").ap()}
    with tile.TileContext(nc) as tc:
        emit(tc, outs, ins)
    nc.compile()
    _CACHE["nc"] = nc
    return nc


def make_in_maps(x, fc1_w, fc1_b, w_ih, w_hh, b_ih, b_hh, fc2_w, fc2_b):
    consts = prep_consts(fc1_w, fc1_b, w_ih, w_hh, b_ih, b_hh, fc2_w, fc2_b)
    in_maps = []
    for c in range(NCORES):
        xs = np.ascontiguousarray(
            x[c * BL : (c + 1) * BL].reshape(BL, CIN * T)
        ).astype(np.float32)
        in_maps.append({"x": xs, **consts})
    return in_maps


def kernel(x, fc1_w, fc1_b, w_ih, w_hh, b_ih, b_hh, fc2_w, fc2_b, trace=False):
    x = np.asarray(x, np.float32)
    args = [
        np.asarray(a, np.float32)
        for a in (fc1_w, fc1_b, w_ih, w_hh, b_ih, b_hh, fc2_w, fc2_b)
    ]
    nc = _build()
    in_maps = make_in_maps(x, *args)
    res = bass_utils.run_bass_kernel_spmd(
        nc, in_maps, core_ids=list(range(NCORES)), trace=trace
    )
    out = np.concatenate([r["out"] for r in res.results], axis=0)
    out = out + args[7][None, :]
    if trace:
        kernel.last_results = res
    return out.astype(np.float32)
